# revision 8
# baseline (speedup 1.0000x reference)
"""Bass/Tile TRN2 kernel for BasicAttention.

att = softmax(tanh(hidden @ W_h.T + p_att_feats) @ W_alpha + mask) @ att_feats

Shapes: B=64, N=2048, H=1024, A=512. Data-parallel over batch across 8
NeuronCores (8 batches per core); W_h / W_alpha replicated; no collectives.

Per-core dataflow (memory-bound: ~96MB HBM reads/core):
  setup: W_h -> PE-transpose -> w_h = hidden @ W_h.T (PE) -> per-batch
         partition-broadcast of w_h rows via ones-matmul.
  per batch b:
    p_att stream [128,4,512]: DVE add (w_h bcast) -> ACT tanh (bf16)
      -> DVE tensor_tensor_reduce vs W_alpha -> scores[128,16]
    scores: + mask, ACT exp (accum rowsum), PE total-sum, DVE reciprocal
    att_feats stream [128,2,1024]: PE f32r matmuls (attn as stationary
      [128,1]) accumulating att[1,1024] in PSUM -> DVE scale by 1/sum -> out.
"""

import numpy as np

B, N, H, A = 64, 2048, 1024, 512
NCORES = 8
BLOC = B // NCORES  # batches per core

P = 128
NT = N // P            # 16 n-chunks of 128
PATT_SUP = 4           # n-chunks per p_att supertile
AF_SUP = 2             # n-chunks per att_feats supertile

_NC_CACHE = {}


def _free_bcast(bass_mod, ap, repeat):
    """[P, F] AP -> [P, repeat, F] AP with 0-stride middle dim."""
    return bass_mod.AP(
        tensor=ap.tensor,
        offset=ap.offset,
        ap=[ap.ap[0], [0, repeat], *ap.ap[1:]],
    )


def _build_nc():
    import concourse.bass as bass
    import concourse.mybir as mybir
    import concourse.tile as tile
    from concourse import bacc
    from concourse.masks import make_identity

    dt = mybir.dt
    f32, f32r, bf16 = dt.float32, dt.float32r, dt.bfloat16
    AF = mybir.ActivationFunctionType
    OP = mybir.AluOpType

    nc = bacc.Bacc("TRN2", target_bir_lowering=False, debug=False,
                   num_devices=NCORES)

    hs = nc.dram_tensor("hidden_states", [BLOC, H], f32, kind="ExternalInput").ap()
    af = nc.dram_tensor("att_feats", [BLOC, N, H], f32, kind="ExternalInput").ap()
    pa = nc.dram_tensor("p_att_feats", [BLOC, N, A], f32, kind="ExternalInput").ap()
    am = nc.dram_tensor("att_masks", [BLOC, N], f32, kind="ExternalInput").ap()
    wh = nc.dram_tensor("W_h", [A, H], f32, kind="ExternalInput").ap()
    wa = nc.dram_tensor("W_alpha", [1, A], f32, kind="ExternalInput").ap()
    out = nc.dram_tensor("att_out", [BLOC, H], f32, kind="ExternalOutput").ap()

    with tile.TileContext(nc) as tc:
        with (
            tc.tile_pool(name="consts", bufs=1) as consts,
            tc.tile_pool(name="patt", bufs=3) as patt_pool,
            tc.tile_pool(name="alpha", bufs=3) as alpha_pool,
            tc.tile_pool(name="afp", bufs=4) as af_pool,
            tc.tile_pool(name="small", bufs=2) as small,
            tc.tile_pool(name="psmisc", bufs=3, space="PSUM") as psmisc,
            tc.tile_pool(name="psatt", bufs=4, space="PSUM") as psatt,
        ):
            # ---------------- setup ----------------
            identity = consts.tile([P, P], f32)
            make_identity(nc, identity)
            ones_row = consts.tile([1, P], f32)
            nc.vector.memset(ones_row, 1.0)
            ones_col = consts.tile([P, 1], f32)
            nc.vector.memset(ones_col, 1.0)

            wh_sb = []
            for at in range(A // P):  # 4 tiles [128a, 1024h]
                t = consts.tile([P, H], f32, name=f"wh{at}", tag=f"wh{at}")
                nc.sync.dma_start(out=t, in_=wh[at * P:(at + 1) * P, :])
                wh_sb.append(t)
            hidden_sb = consts.tile([BLOC, H], f32)
            nc.sync.dma_start(out=hidden_sb, in_=hs[:, :])
            wa_sb = consts.tile([1, A], f32)
            nc.sync.dma_start(out=wa_sb, in_=wa[:, :])

            # W_hT tiles: [128h, 512a] per h-chunk (8)
            whT = [consts.tile([P, A], f32, name=f"whT{hc}", tag=f"whT{hc}")
                   for hc in range(H // P)]
            for at in range(A // P):
                for hc in range(H // P):
                    pt = psmisc.tile([P, P], f32, tag="mm")
                    nc.tensor.transpose(pt, wh_sb[at][:, hc * P:(hc + 1) * P], identity)
                    nc.vector.tensor_copy(whT[hc][:, at * P:(at + 1) * P], pt)
            # hiddenT tiles: [128h, 8b] per h-chunk
            hidT = [consts.tile([P, BLOC], f32, name=f"hidT{hc}", tag=f"hidT{hc}")
                    for hc in range(H // P)]
            for hc in range(H // P):
                pt = psmisc.tile([P, BLOC], f32, tag="mm")
                nc.tensor.transpose(
                    pt, hidden_sb[0:BLOC, hc * P:(hc + 1) * P], identity[0:BLOC, 0:BLOC]
                )
                nc.vector.tensor_copy(hidT[hc], pt)

            # w_h = hidden @ W_h.T : [8, 512]
            wh_ps = psmisc.tile([BLOC, A], f32, tag="mm")
            for hc in range(H // P):
                nc.tensor.matmul(wh_ps, lhsT=hidT[hc], rhs=whT[hc],
                                 start=(hc == 0), stop=(hc == H // P - 1))
            whall_sb = consts.tile([BLOC, A], f32)
            nc.vector.tensor_copy(whall_sb, wh_ps)

            # W_alpha broadcast to [128, 512] bf16
            wab_ps = psmisc.tile([P, A], f32, tag="mm")
            nc.tensor.matmul(wab_ps, lhsT=ones_row, rhs=wa_sb, start=True, stop=True)
            wa_bf = consts.tile([P, A], bf16)
            nc.vector.tensor_copy(wa_bf, wab_ps)

            # per-batch w_h row broadcast to [128, 512] f32 via a DRAM
            # round-trip with a 0-stride partition AP (setup-only, ~2MB)
            whall_dram = nc.dram_tensor("whall_scratch", [BLOC, A], f32).ap()
            nc.sync.dma_start(out=whall_dram, in_=whall_sb)
            whb = []
            for b in range(BLOC):
                t = consts.tile([P, A], f32, name=f"whb{b}", tag=f"whb{b}")
                row = whall_dram[b:b + 1, :]
                src = bass.AP(tensor=row.tensor, offset=row.offset,
                              ap=[[0, P], row.ap[1]])
                nc.sync.dma_start(out=t, in_=src)
                whb.append(t)

            # ---------------- main loop ----------------
            for b in range(BLOC):
                scores = small.tile([P, NT], f32, tag="scores")
                for st in range(NT // PATT_SUP):  # 4 supertiles
                    pt = patt_pool.tile([P, PATT_SUP, A], f32, tag="patt")
                    nc.sync.dma_start(
                        out=pt,
                        in_=pa[b, st * PATT_SUP * P:(st + 1) * PATT_SUP * P, :]
                        .rearrange("(c p) a -> p c a", p=P),
                    )
                    whb_b = _free_bcast(bass, whb[b][:, :], PATT_SUP)
                    nc.vector.tensor_tensor(out=pt, in0=pt, in1=whb_b, op=OP.add)
                    ab = alpha_pool.tile([P, PATT_SUP, A], bf16, tag="alpha")
                    nc.scalar.activation(ab, pt, AF.Tanh)
                    for c in range(PATT_SUP):
                        col = st * PATT_SUP + c
                        # out = (ab * 1) * wa ; accum_out = row-sum -> scores
                        nc.vector.scalar_tensor_tensor(
                            out=ab[:, c, :], in0=ab[:, c, :], scalar=1.0,
                            in1=wa_bf, op0=OP.mult, op1=OP.mult,
                            accum_out=scores[:, col:col + 1],
                        )

                masks = small.tile([P, NT], f32, tag="masks")
                nc.sync.dma_start(out=masks, in_=am[b, :].rearrange("(t p) -> p t", p=P))
                nc.vector.tensor_tensor(out=scores, in0=scores, in1=masks, op=OP.add)

                expt = small.tile([P, NT], f32, tag="expt")
                rowsum = small.tile([P, 1], f32, tag="rowsum")
                nc.scalar.activation(expt, scores, AF.Exp, accum_out=rowsum)

                sum_ps = psmisc.tile([1, 1], f32, tag="mm")
                nc.tensor.matmul(sum_ps, lhsT=rowsum, rhs=ones_col,
                                 start=True, stop=True)
                inv = small.tile([1, 1], f32, tag="inv")
                nc.vector.reciprocal(inv, sum_ps)

                att_lo = psatt.tile([1, A], f32, tag="att")
                att_hi = psatt.tile([1, A], f32, tag="att")
                for st2 in range(NT // AF_SUP):  # 8 supertiles
                    aft = af_pool.tile([P, AF_SUP, H], f32, tag="af")
                    nc.sync.dma_start(
                        out=aft,
                        in_=af[b, st2 * AF_SUP * P:(st2 + 1) * AF_SUP * P, :]
                        .rearrange("(c p) h -> p c h", p=P),
                    )
                    for c in range(AF_SUP):
                        t = st2 * AF_SUP + c
                        lhs = expt[:, t:t + 1]
                        nc.tensor.matmul(att_lo, lhsT=lhs,
                                         rhs=aft[:, c, 0:A],
                                         start=(t == 0), stop=(t == NT - 1))
                        nc.tensor.matmul(att_hi, lhsT=lhs,
                                         rhs=aft[:, c, A:H],
                                         start=(t == 0), stop=(t == NT - 1))

                att_row = small.tile([1, H], f32, tag="attrow")
                nc.vector.tensor_scalar_mul(att_row[:, 0:A], att_lo, inv)
                nc.vector.tensor_scalar_mul(att_row[:, A:H], att_hi, inv)
                nc.sync.dma_start(out=out[b:b + 1, :], in_=att_row)

    nc.compile()
    return nc


def _get_nc():
    if "nc" not in _NC_CACHE:
        _NC_CACHE["nc"] = _build_nc()
    return _NC_CACHE["nc"]


def kernel(hidden_states, att_feats, p_att_feats, att_masks, W_h, W_alpha):
    from concourse.bass_utils import run_bass_kernel_spmd

    nc = _get_nc()
    hidden_states = np.ascontiguousarray(hidden_states, dtype=np.float32)
    att_feats = np.ascontiguousarray(att_feats, dtype=np.float32)
    p_att_feats = np.ascontiguousarray(p_att_feats, dtype=np.float32)
    att_masks = np.ascontiguousarray(att_masks, dtype=np.float32)
    W_h = np.ascontiguousarray(W_h, dtype=np.float32)
    W_alpha = np.ascontiguousarray(W_alpha, dtype=np.float32).reshape(1, A)

    in_maps = []
    for i in range(NCORES):
        s = slice(i * BLOC, (i + 1) * BLOC)
        in_maps.append({
            "hidden_states": hidden_states[s],
            "att_feats": att_feats[s],
            "p_att_feats": p_att_feats[s],
            "att_masks": att_masks[s],
            "W_h": W_h,
            "W_alpha": W_alpha,
        })

    global _LAST_IN_MAPS
    _LAST_IN_MAPS = in_maps
    res = run_bass_kernel_spmd(nc, in_maps, core_ids=list(range(NCORES)))
    return np.concatenate(
        [res.results[i]["att_out"] for i in range(NCORES)], axis=0
    ).astype(np.float32)


_LAST_IN_MAPS = None


# revision 9
# speedup vs baseline: 1.0611x; 1.0611x over previous
"""Bass/Tile TRN2 kernel for BasicAttention.

att = softmax(tanh(hidden @ W_h.T + p_att_feats) @ W_alpha + mask) @ att_feats

Shapes: B=64, N=2048, H=1024, A=512. Data-parallel over batch across 8
NeuronCores (8 batches per core); W_h / W_alpha replicated; no collectives.

Per-core dataflow (memory-bound: ~96MB HBM reads/core):
  setup: W_h -> PE-transpose -> w_h = hidden @ W_h.T (PE) -> per-batch
         partition-broadcast of w_h rows via ones-matmul.
  per batch b:
    p_att stream [128,4,512]: DVE add (w_h bcast) -> ACT tanh (bf16)
      -> DVE tensor_tensor_reduce vs W_alpha -> scores[128,16]
    scores: + mask, ACT exp (accum rowsum), PE total-sum, DVE reciprocal
    att_feats stream [128,2,1024]: PE f32r matmuls (attn as stationary
      [128,1]) accumulating att[1,1024] in PSUM -> DVE scale by 1/sum -> out.
"""

import numpy as np

B, N, H, A = 64, 2048, 1024, 512
NCORES = 8
BLOC = B // NCORES  # batches per core

P = 128
NT = N // P            # 16 n-chunks of 128
PATT_SUP = 4           # n-chunks per p_att supertile
AF_SUP = 2             # n-chunks per att_feats supertile

_NC_CACHE = {}


def _free_bcast(bass_mod, ap, repeat):
    """[P, F] AP -> [P, repeat, F] AP with 0-stride middle dim."""
    return bass_mod.AP(
        tensor=ap.tensor,
        offset=ap.offset,
        ap=[ap.ap[0], [0, repeat], *ap.ap[1:]],
    )


def _build_nc():
    import concourse.bass as bass
    import concourse.mybir as mybir
    import concourse.tile as tile
    from concourse import bacc
    from concourse.masks import make_identity

    dt = mybir.dt
    f32, f32r, bf16 = dt.float32, dt.float32r, dt.bfloat16
    AF = mybir.ActivationFunctionType
    OP = mybir.AluOpType

    nc = bacc.Bacc("TRN2", target_bir_lowering=False, debug=False,
                   num_devices=NCORES)

    hs = nc.dram_tensor("hidden_states", [BLOC, H], f32, kind="ExternalInput").ap()
    af = nc.dram_tensor("att_feats", [BLOC, N, H], f32r, kind="ExternalInput").ap()
    pa = nc.dram_tensor("p_att_feats", [BLOC, N, A], f32, kind="ExternalInput").ap()
    am = nc.dram_tensor("att_masks", [BLOC, N], f32, kind="ExternalInput").ap()
    wh = nc.dram_tensor("W_h", [A, H], f32, kind="ExternalInput").ap()
    wa = nc.dram_tensor("W_alpha", [1, A], f32, kind="ExternalInput").ap()
    out = nc.dram_tensor("att_out", [BLOC, H], f32, kind="ExternalOutput").ap()

    with tile.TileContext(nc) as tc:
        with (
            tc.tile_pool(name="consts", bufs=1) as consts,
            tc.tile_pool(name="patt", bufs=3) as patt_pool,
            tc.tile_pool(name="alpha", bufs=3) as alpha_pool,
            tc.tile_pool(name="afp", bufs=4) as af_pool,
            tc.tile_pool(name="small", bufs=2) as small,
            tc.tile_pool(name="psmisc", bufs=3, space="PSUM") as psmisc,
            tc.tile_pool(name="psatt", bufs=4, space="PSUM") as psatt,
        ):
            # ---------------- setup ----------------
            identity = consts.tile([P, P], f32)
            make_identity(nc, identity)
            ones_row = consts.tile([1, P], f32)
            nc.vector.memset(ones_row, 1.0)
            ones_col = consts.tile([P, 1], f32)
            nc.vector.memset(ones_col, 1.0)

            wh_sb = []
            for at in range(A // P):  # 4 tiles [128a, 1024h]
                t = consts.tile([P, H], f32, name=f"wh{at}", tag=f"wh{at}")
                nc.sync.dma_start(out=t, in_=wh[at * P:(at + 1) * P, :])
                wh_sb.append(t)
            hidden_sb = consts.tile([BLOC, H], f32)
            nc.sync.dma_start(out=hidden_sb, in_=hs[:, :])
            wa_sb = consts.tile([1, A], f32)
            nc.sync.dma_start(out=wa_sb, in_=wa[:, :])

            # W_hT tiles: [128h, 512a] per h-chunk (8)
            whT = [consts.tile([P, A], f32, name=f"whT{hc}", tag=f"whT{hc}")
                   for hc in range(H // P)]
            for at in range(A // P):
                for hc in range(H // P):
                    pt = psmisc.tile([P, P], f32, tag="mm")
                    nc.tensor.transpose(pt, wh_sb[at][:, hc * P:(hc + 1) * P], identity)
                    nc.vector.tensor_copy(whT[hc][:, at * P:(at + 1) * P], pt)
            # hiddenT tiles: [128h, 8b] per h-chunk
            hidT = [consts.tile([P, BLOC], f32, name=f"hidT{hc}", tag=f"hidT{hc}")
                    for hc in range(H // P)]
            for hc in range(H // P):
                pt = psmisc.tile([P, BLOC], f32, tag="mm")
                nc.tensor.transpose(
                    pt, hidden_sb[0:BLOC, hc * P:(hc + 1) * P], identity[0:BLOC, 0:BLOC]
                )
                nc.vector.tensor_copy(hidT[hc], pt)

            # w_h = hidden @ W_h.T : [8, 512]
            wh_ps = psmisc.tile([BLOC, A], f32, tag="mm")
            for hc in range(H // P):
                nc.tensor.matmul(wh_ps, lhsT=hidT[hc], rhs=whT[hc],
                                 start=(hc == 0), stop=(hc == H // P - 1))
            whall_sb = consts.tile([BLOC, A], f32)
            nc.vector.tensor_copy(whall_sb, wh_ps)

            # W_alpha broadcast to [128, 512] bf16
            wab_ps = psmisc.tile([P, A], f32, tag="mm")
            nc.tensor.matmul(wab_ps, lhsT=ones_row, rhs=wa_sb, start=True, stop=True)
            wa_bf = consts.tile([P, A], bf16)
            nc.vector.tensor_copy(wa_bf, wab_ps)

            # per-batch w_h row broadcast to [128, 512] f32 via a DRAM
            # round-trip with a 0-stride partition AP (setup-only, ~2MB)
            whall_dram = nc.dram_tensor("whall_scratch", [BLOC, A], f32).ap()
            nc.sync.dma_start(out=whall_dram, in_=whall_sb)
            whb = []
            for b in range(BLOC):
                t = consts.tile([P, A], f32, name=f"whb{b}", tag=f"whb{b}")
                row = whall_dram[b:b + 1, :]
                src = bass.AP(tensor=row.tensor, offset=row.offset,
                              ap=[[0, P], row.ap[1]])
                nc.sync.dma_start(out=t, in_=src)
                whb.append(t)

            # ---------------- main loop ----------------
            for b in range(BLOC):
                scores = small.tile([P, NT], f32, tag="scores")
                for st in range(NT // PATT_SUP):  # 4 supertiles
                    pt = patt_pool.tile([P, PATT_SUP, A], f32, tag="patt")
                    nc.sync.dma_start(
                        out=pt,
                        in_=pa[b, st * PATT_SUP * P:(st + 1) * PATT_SUP * P, :]
                        .rearrange("(c p) a -> p c a", p=P),
                    )
                    whb_b = _free_bcast(bass, whb[b][:, :], PATT_SUP)
                    nc.vector.tensor_tensor(out=pt, in0=pt, in1=whb_b, op=OP.add)
                    ab = alpha_pool.tile([P, PATT_SUP, A], bf16, tag="alpha")
                    nc.scalar.activation(ab, pt, AF.Tanh)
                    for c in range(PATT_SUP):
                        col = st * PATT_SUP + c
                        # out = (ab * 1) * wa ; accum_out = row-sum -> scores
                        nc.vector.scalar_tensor_tensor(
                            out=ab[:, c, :], in0=ab[:, c, :], scalar=1.0,
                            in1=wa_bf, op0=OP.mult, op1=OP.mult,
                            accum_out=scores[:, col:col + 1],
                        )

                masks = small.tile([P, NT], f32, tag="masks")
                nc.sync.dma_start(out=masks, in_=am[b, :].rearrange("(t p) -> p t", p=P))
                nc.vector.tensor_tensor(out=scores, in0=scores, in1=masks, op=OP.add)

                expt = small.tile([P, NT], f32r, tag="expt")
                rowsum = small.tile([P, 1], f32, tag="rowsum")
                nc.scalar.activation(expt, scores, AF.Exp, accum_out=rowsum)

                sum_ps = psmisc.tile([1, 1], f32, tag="mm")
                nc.tensor.matmul(sum_ps, lhsT=rowsum, rhs=ones_col,
                                 start=True, stop=True)
                inv = small.tile([1, 1], f32, tag="inv")
                nc.vector.reciprocal(inv, sum_ps)

                att_lo = psatt.tile([1, A], f32, tag="att")
                att_hi = psatt.tile([1, A], f32, tag="att")
                for st2 in range(NT // AF_SUP):  # 8 supertiles
                    aft = af_pool.tile([P, AF_SUP, H], f32r, tag="af")
                    nc.sync.dma_start(
                        out=aft,
                        in_=af[b, st2 * AF_SUP * P:(st2 + 1) * AF_SUP * P, :]
                        .rearrange("(c p) h -> p c h", p=P),
                    )
                    for c in range(AF_SUP):
                        t = st2 * AF_SUP + c
                        lhs = expt[:, t:t + 1]
                        nc.tensor.matmul(att_lo, lhsT=lhs,
                                         rhs=aft[:, c, 0:A],
                                         start=(t == 0), stop=(t == NT - 1))
                        nc.tensor.matmul(att_hi, lhsT=lhs,
                                         rhs=aft[:, c, A:H],
                                         start=(t == 0), stop=(t == NT - 1))

                att_row = small.tile([1, H], f32, tag="attrow")
                nc.vector.tensor_scalar_mul(att_row[:, 0:A], att_lo, inv)
                nc.vector.tensor_scalar_mul(att_row[:, A:H], att_hi, inv)
                nc.sync.dma_start(out=out[b:b + 1, :], in_=att_row)

    nc.compile()
    return nc


def _get_nc():
    if "nc" not in _NC_CACHE:
        _NC_CACHE["nc"] = _build_nc()
    return _NC_CACHE["nc"]


def kernel(hidden_states, att_feats, p_att_feats, att_masks, W_h, W_alpha):
    from concourse.bass_utils import run_bass_kernel_spmd

    nc = _get_nc()
    hidden_states = np.ascontiguousarray(hidden_states, dtype=np.float32)
    att_feats = np.ascontiguousarray(att_feats, dtype=np.float32)
    p_att_feats = np.ascontiguousarray(p_att_feats, dtype=np.float32)
    att_masks = np.ascontiguousarray(att_masks, dtype=np.float32)
    W_h = np.ascontiguousarray(W_h, dtype=np.float32)
    W_alpha = np.ascontiguousarray(W_alpha, dtype=np.float32).reshape(1, A)

    in_maps = []
    for i in range(NCORES):
        s = slice(i * BLOC, (i + 1) * BLOC)
        in_maps.append({
            "hidden_states": hidden_states[s],
            "att_feats": att_feats[s],
            "p_att_feats": p_att_feats[s],
            "att_masks": att_masks[s],
            "W_h": W_h,
            "W_alpha": W_alpha,
        })

    global _LAST_IN_MAPS
    _LAST_IN_MAPS = in_maps
    res = run_bass_kernel_spmd(nc, in_maps, core_ids=list(range(NCORES)))
    return np.concatenate(
        [res.results[i]["att_out"] for i in range(NCORES)], axis=0
    ).astype(np.float32)


_LAST_IN_MAPS = None
